# revision 26
# baseline (speedup 1.0000x reference)
"""Multi-head attention (query-axis softmax variant) on 8 Trainium2 NeuronCores.

Problem: B=4, T=2048, C=1024, H=16, Dh=64.
  q/k/v = per-head projections of x; wei = (q k^T) * C**-0.5, causal-masked;
  softmax over the QUERY axis (axis=2 of (B,H,T,S)); out = attn @ v, concat
  heads, project with Wp and add bp.

Sharding: 8 cores = 4 batches x 2 head-groups (8 heads each).  Each core
computes a partial projection output for its batch; host sums the two
group partials per batch and adds the bias.

Per-core dataflow is fully "transposed" (features on partitions, tokens on
the free axis) so the query-axis softmax stats become free-axis reductions:
  xT (C, T) -> qT/kT per head-pair (128, T) -> scores W[s,t] per key-tile
  -> P = exp(W*scale) with the masked entries driven to 0 via a -1e30
  additive triangle, Z[s] = accumulated row sums from the Exp activation
  -> v' = v * (1/Z) -> attout^T[d,t] -> y = attout^T.T @ WpT.

All matmuls run in bf16.  The two heads of a pair are interleaved at chunk
granularity: score matmuls alternate PE row groups (64-row halves) and
attout matmuls alternate column groups, so each chunk's pair of matmuls
executes concurrently in the PE array and the next LDWEIGHTS pulls ahead
under the other group's matmul.  The attention loop is software-pipelined
(scores one iteration ahead, attout two behind) with the q/k projections of
the next pair, the v projection and output-projection groups interleaved as
PE filler.  Softmax stats are batched per iteration: one [P,4] accum tile,
one gpsimd add + one DVE reciprocal for both heads, v*(1/Z) on gpsimd.
"""
import numpy as np

T = 2048
C = 1024
H = 16
DH = 64
B = 4
SCALE = float(C) ** -0.5
P = 128

# Wq is pre-scaled on the host by QS = SCALE/(16*sqrt2), so the score psum
# holds x' = QS * (q.k).  exp(SCALE*q.k) is then
#   ACT path:  exp(x' * 16*sqrt2)
#   DVE path:  ((x' + 1/sqrt2)^2 + 1/2)^16    [exp16 poly, see below]
# The causal mask adds -1/sqrt2 to x' (i.e. raw score -512): ACT gives
# e^-16 ~ 1e-7, the poly hits its global minimum (1/2)^16 ~ 1.5e-5 -- both
# effectively zero, and the poly cannot overflow (a -1e30 mask would).
import math as _math
QS = SCALE / (16.0 * _math.sqrt(2.0))
ACT_SCALE = 16.0 * _math.sqrt(2.0)
NEG = -1.0 / _math.sqrt(2.0)

_CACHE = {}


def _register_exp16():
    """Register a runtime custom-DVE op computing exp-of-score with a Z
    accumulator, so the Vector engine can take half the softmax exps.

    exp(y) ~ (1 + b + b^2/2)^16 with b = y/16, via the identity
    1 + b + b^2/2 = ((b+1)/sqrt2)^2 + 1/2.  With the input pre-scaled to
    x' = y/(16*sqrt2) this is 7 ALU stages + 1 accum stage:
      t = x' + C0;  v = t^2 + C1;  out = (((v^2)^2)^2)^2
    with C0 = 1/sqrt2, C1 = 1/2.  Max relative error ~0.22% over the
    realized score range |y|<1.5, inside the bf16 output quantization."""
    import concourse.dve_ops as dve_ops
    from concourse.dve_spec import Spec, Src0, C0, C1, sq, lower
    from concourse.dve_uop import DveOpSpec
    from operator import add as _add

    for op in dve_ops.OPS:
        if op.name == "EXP16_ANT":
            return op
    body = sq(sq(sq(sq(sq(Src0 + C0) + C1))))
    spec = Spec(body=body, accum=_add)
    shas = {}
    for ver in ("v3", "v4"):
        tmp = DveOpSpec(name="EXP16_ANT", uops=lower(spec, ver=ver), rd1_en=False)
        shas[ver] = tmp.sha(ver)
    op = dve_ops.DveOp("EXP16_ANT", spec, False, shas)
    dve_ops.OPS.append(op)
    dve_ops._SUB_OPCODE_FOR_NAME[op.name] = (
        dve_ops._CUSTOM_DVE_ROW_BASE + len(dve_ops.OPS) - 1)
    dve_ops.CUSTOM_DVE_SPECS[op.name] = spec
    return op


def _build_nc():
    import concourse.bacc as bacc
    import concourse.tile as tile
    import concourse.mybir as mybir
    from contextlib import ExitStack

    FP = mybir.dt.float32
    BF = mybir.dt.bfloat16
    EXP = mybir.ActivationFunctionType.Exp
    EXP16 = _register_exp16()
    E16_S0 = 1.0 / _math.sqrt(2.0)
    E16_S1 = 0.5

    nc = bacc.Bacc("TRN2", target_bir_lowering=False, debug=False, num_devices=8)

    xT_d = nc.declare_dram_parameter("xt", [C, T], BF, isOutput=False)
    wq_d = nc.declare_dram_parameter("wq", [C, 512], BF, isOutput=False)
    wk_d = nc.declare_dram_parameter("wk", [C, 512], BF, isOutput=False)
    wv_d = nc.declare_dram_parameter("wv", [C, 512], BF, isOutput=False)
    wp_d = nc.declare_dram_parameter("wpt", [512, C], BF, isOutput=False)
    tri_d = nc.declare_dram_parameter("tri", [P, P], FP, isOutput=False)
    y_d = nc.declare_dram_parameter("y", [T, C], FP, isOutput=True)

    NCT = C // P      # 8 c-tiles
    NST = T // P      # 16 s-tiles
    NTG = T // 512    # 4 t-groups

    with tile.TileContext(nc) as tc:
        with (
            tc.tile_pool(name="perm", bufs=1) as perm,
            tc.tile_pool(name="work", bufs=4) as work,
            tc.tile_pool(name="stat", bufs=4) as stat,
            tc.tile_pool(name="statv", bufs=4) as statv,
            tc.tile_pool(name="pao", bufs=1) as pao,
            tc.tile_pool(name="ps", bufs=2, space="PSUM") as pspool,
            tc.tile_pool(name="avps", bufs=4, space="PSUM") as avpool,
        ):
            tri = perm.tile([P, P], FP, tag="tri")
            nc.sync.dma_start(tri[:], tri_d[:])
            v_sb = perm.tile([P, NST, 512], BF, tag="v")
            q_sb = perm.tile([P, 4, T], BF, tag="q")
            k_sb = perm.tile([P, 4, T], BF, tag="k")

            es = ExitStack()
            pxw = es.enter_context(tc.tile_pool(name="px", bufs=1))
            wpool = es.enter_context(tc.tile_pool(name="w", bufs=2))

            # warm tile + early activation-table load (overlaps input DMAs)
            warm = perm.tile([P, 512], BF, tag="warm")
            nc.vector.memset(warm[:], 0.0)
            wexp = perm.tile([P, 1], BF, tag="wexp")
            nc.scalar.activation(wexp[:], warm[:, 0:1], EXP, scale=SCALE)

            # ---- phase 1: qT/kT per pair (128 = [h0 d, h1 d], T), bf16 out ----
            xT = pxw.tile([P, NCT, T], BF, tag="xT")
            wv = pxw.tile([P, NCT, 512], BF, tag="wv")

            def emit_qk_steps(p):
                """Returns a list of closures; each emits one 512-col psum group."""
                wqt = wpool.tile([P, NCT, P], BF, tag="wq")
                wkt = wpool.tile([P, NCT, P], BF, tag="wk")
                nc.sync.dma_start(
                    wqt[:], wq_d.ap()[:, P * p:P * p + P].rearrange("(a c) m -> c a m", c=P))
                nc.sync.dma_start(
                    wkt[:], wk_d.ap()[:, P * p:P * p + P].rearrange("(a c) m -> c a m", c=P))
                steps = []
                for g in range(NTG):
                    for wt, dst in ((wqt, q_sb), (wkt, k_sb)):
                        def step(wt=wt, dst=dst, g=g, p=p, pool=None):
                            pl = pool if pool is not None else pspool
                            psq = pl.tile([P, 512], FP, tag="av" if pool is not None else "ps",
                                          name=f"qkps{p}_{g}_{dst is k_sb}")
                            for ct in range(NCT):
                                nc.tensor.matmul(
                                    psq[:, :512], lhsT=wt[:, ct, :],
                                    rhs=xT[:, ct, 512 * g:512 * g + 512],
                                    start=(ct == 0), stop=(ct == NCT - 1))
                            nc.scalar.copy(dst[:, p, 512 * g:512 * g + 512], psq[:, :512])
                        steps.append(step)
                return steps

            # DMA priority: pair-0 q/k weights first (the prefix needs them
            # immediately), then xT by t-group with the v weights mid-stream.
            steps0 = emit_qk_steps(0)
            for g in range(NTG):
                # per (t-group, c-half) so the accumulation matmuls can pace
                # with the DMA arrivals at half-group granularity
                for ch in range(2):
                    nc.sync.dma_start(
                        xT[:, 4 * ch:4 * ch + 4, 512 * g:512 * g + 512],
                        xT_d.ap()[512 * ch:512 * ch + 512,
                                  512 * g:512 * g + 512].rearrange(
                            "(a c) t -> c a t", c=P))
                if g == 1:
                    nc.sync.dma_start(wv[:], wv_d.ap().rearrange("(a c) m -> c a m", c=P))

            # Warm up the PE's HAM clock gate while the input DMAs land (kept
            # short: these sit ahead of the real work in the in-order queue).
            for wi in range(8):
                wps = pspool.tile([P, 1024], FP, tag="ps", name=f"warm{wi}")
                for _ in range(2):
                    nc.tensor.matmul(wps[:, :512], lhsT=warm[:, :P], rhs=warm[:],
                                     start=True, stop=True)

            def v_step(st, pool=None):
                pl = pool if pool is not None else pspool
                ps = pl.tile([P, 512] if pool is not None else [P, 1024], FP,
                             tag="av" if pool is not None else "ps", name=f"vps{st}")
                for ct in range(NCT):
                    nc.tensor.matmul(
                        ps[:, :512],
                        lhsT=xT[:, ct, P * st:P * st + P],
                        rhs=wv[:, ct, :],
                        start=(ct == 0), stop=(ct == NCT - 1))
                nc.scalar.copy(v_sb[:, st, :], ps[:, :512])

            # ---- serial prefix: pair-0 q/k projections.  scores(0,0) is
            # emitted after the first four groups (its block A only needs
            # q groups 0-1 and k group 0) so the exps start ~9us earlier;
            # the remaining q groups come before k groups 2-3 (block B of
            # scores 0-1 needs q, while k tiles 2-3 are only read at i>=8).
            # ---- phase 2: attention ----
            ao = pao.tile([P, 4, T], BF, tag="ao")

            late_sps = {}

            def emit_scores(p, i):
                """PE: both heads' score matmuls, chunk-interleaved so the two
                row-group halves run concurrently; DVE: diag masks.

                For i>=12 the block is <=512 cols, so iterations (12,13) and
                (14,15) share one [P,1024] tile per head at column offsets
                0/512: scores(i+1) then never waits on exp(i) (different psum
                banks of the already-granted slot).  For i>=8 the h1 chunk is
                emitted first: its slot is freed by the DVE exp16, which
                finishes earlier than the ACT exp that frees h0's slot."""
                t0 = P * i
                blocks = [(t0, 1024), (1024, 2048)] if i < 8 else [(t0, 2048)]
                prows = [work.tile([P, T], BF, tag="prow", bufs=8,
                                   name=f"prow{p}_{i}_{h}") for h in range(2)]
                zps = stat.tile([P, 4], FP, tag="zp", bufs=8, name=f"zp{p}_{i}")
                hls = (1, 0) if i >= 8 else (0, 1)
                tiles = []
                for bi, (lo, hi) in enumerate(blocks):
                    if i >= 12:
                        if i % 2 == 0:
                            late_sps[(p, i // 2)] = [
                                pspool.tile([P, 1024], FP, tag="ps",
                                            name=f"sps{p}_{i}_{h}") for h in range(2)]
                        base = late_sps[(p, i // 2)]
                        off = 512 * (i % 2)
                    else:
                        base = [pspool.tile([P, 1024], FP, tag="ps",
                                            name=f"sps{p}_{i}_{bi}_{h}") for h in range(2)]
                        off = 0
                    for clo in range(lo, hi, 512):
                        chi = min(clo + 512, hi)
                        for hl in hls:
                            hb = 64 * hl
                            nc.tensor.matmul(
                                base[hl][:, off + clo - lo:off + chi - lo],
                                lhsT=k_sb[hb:hb + 64, p, t0:t0 + P],
                                rhs=q_sb[hb:hb + 64, p, clo:chi],
                                start=True, stop=True)
                        if clo == t0:
                            for hl in hls:
                                nc.vector.tensor_add(
                                    base[hl][:, off:off + P],
                                    base[hl][:, off:off + P], tri[:])
                    tiles.append((base, off, bi, lo, hi))
                return dict(i=i, t0=t0, nb=len(blocks), prows=prows, zps=zps, tiles=tiles)

            def emit_exps(sc):
                # Engine split chosen so the two psum ring slots (slot A = h0,
                # slot B = h1) free at ~the same time: each slot gets one ACT
                # and one DVE block when there are two blocks, and the two
                # heads go to different engines when there is one.
                for (base, off, bi, lo, hi) in sc["tiles"]:
                    for hl in range(2):
                        on_dve = (hl ^ bi) if sc["nb"] == 2 else hl
                        zcol = sc["zps"][:, 2 * bi + hl:2 * bi + hl + 1]
                        if on_dve:
                            nc.vector._custom_dve(
                                EXP16, out=sc["prows"][hl][:, lo:hi],
                                in0=base[hl][:, off:off + hi - lo],
                                s0=E16_S0, s1=E16_S1, imm2=0.0, accum_out=zcol)
                        else:
                            nc.scalar.activation(
                                sc["prows"][hl][:, lo:hi],
                                base[hl][:, off:off + hi - lo],
                                EXP, scale=ACT_SCALE, accum_out=zcol)

            def emit_stats(p, sc):
                i = sc["i"]
                zps = sc["zps"]
                if sc["nb"] == 2:
                    zz = stat.tile([P, 2], FP, tag="z", name=f"z{p}_{i}")
                    nc.gpsimd.tensor_add(zz[:], zps[:, 0:2], zps[:, 2:4])
                    zzap = zz[:]
                else:
                    zzap = zps[:, 0:2]
                rz = stat.tile([P, 2], FP, tag="rz", name=f"rz{p}_{i}")
                nc.vector.reciprocal(rz[:], zzap)
                vps = []
                for hl in range(2):
                    vp = statv.tile([P, 64], BF, tag="vp", bufs=6, name=f"vp{p}_{i}_{hl}")
                    hh = 64 * (2 * p + hl)
                    nc.vector.tensor_scalar_mul(
                        vp[:], v_sb[:, i, hh:hh + 64], rz[:, hl:hl + 1])
                    vps.append(vp)
                return vps

            wpt = pao.tile([P, 4, C], BF, tag="wpt")
            nc.sync.dma_start(wpt[:], wp_d.ap().rearrange("(a c) m -> c a m", c=P))

            def proj_group(tt, nb, pool=None, on_dve=False):
                pl = pool if pool is not None else avpool
                ps = pl.tile([P, 512], FP, tag="av" if pl is avpool else "ps",
                             name=f"pps{tt}_{nb}")
                for pp in range(4):
                    nc.tensor.matmul(
                        ps[:, :512], lhsT=ao[:, pp, P * tt:P * tt + P],
                        rhs=wpt[:, pp, 512 * nb:512 * nb + 512],
                        start=(pp == 0), stop=(pp == 3))
                yt = work.tile([P, 512], FP, tag="yt", bufs=4, name=f"yt{tt}_{nb}")
                if on_dve:
                    nc.vector.tensor_copy(yt[:], ps[:, :512])
                else:
                    nc.scalar.copy(yt[:], ps[:, :512])
                nc.sync.dma_start(y_d.ap()[P * tt:P * tt + P, 512 * nb:512 * nb + 512], yt[:])

            for si in (0, 1, 2, 3, 4, 6):
                steps0[si]()
            # all q groups are now emitted, so scores(0,0) [needs q 0-3 and
            # k group 0] can be emitted; k groups 2-3 (read at i>=8) follow.
            sc00 = emit_scores(0, 0)
            for si in (5, 7):
                steps0[si]()

            sc0 = sc00
            avc_next = None
            for p in range(4):
                if p == 0:
                    fill = [(lambda st=st: (lambda pool=None: v_step(st, pool=pool)))()
                            for st in range(8, NST)] + emit_qk_steps(1)
                elif p < 3:
                    fill = emit_qk_steps(p + 1)
                else:
                    fill = [(lambda tt=tt, nb=nb: (lambda pool=None: proj_group(tt, nb)))()
                            for tt in range(12) for nb in range(2)]
                if avc_next is None:
                    avc = [avpool.tile([P, 512], FP, tag="av", name=f"avc{p}_{c}")
                           for c in range(NTG)]
                else:
                    avc = avc_next
                avc_next = None
                pend = []
                done_av = -1
                evacd = 0
                sc = sc0 if sc0 is not None else emit_scores(p, 0)
                for i in range(NST):
                    sc_next = emit_scores(p, i + 1) if i < NST - 1 else None
                    emit_exps(sc)
                    if len(pend) >= 2 or (i == NST - 1 and pend):
                        pend.pop(0)()
                        done_av += 1
                    # evacuate finished attout chunks (frees their psum bank
                    # for the qk / projection filler steps); alternate the
                    # copy engine to balance Scalar vs Vector load
                    if evacd < 3 and done_av == 4 * evacd + 3:
                        if evacd % 2:
                            nc.scalar.copy(
                                ao[:, p, 512 * evacd:512 * evacd + 512], avc[evacd][:])
                        else:
                            nc.vector.tensor_copy(
                                ao[:, p, 512 * evacd:512 * evacd + 512], avc[evacd][:])
                        evacd += 1
                    if p == 0 and i < 4:
                        v_step(2 * i)
                        v_step(2 * i + 1)
                    if fill and i >= 6:
                        n = 2 if len(fill) > (NST - 1 - i) else 1
                        for _ in range(min(n, len(fill))):
                            fill.pop(0)(pool=avpool)
                    vps = emit_stats(p, sc)

                    def make_av(i=i, t0=P * i, vps=vps, prows=sc["prows"]):
                        def emit():
                            for c in range(NTG):
                                clo, chi = 512 * c, 512 * c + 512
                                lo2 = max(clo, t0)
                                if lo2 >= chi:
                                    continue
                                for hl in range(2):
                                    hb = 64 * hl
                                    nc.tensor.matmul(
                                        avc[c][hb:hb + 64, lo2 - clo:512],
                                        lhsT=vps[hl][:], rhs=prows[hl][:, lo2:chi],
                                        start=(i == 0), stop=(i == 4 * c + 3))
                        return emit

                    pend.append(make_av())
                    sc = sc_next
                # pair tail: drain attout (ready soonest) and leftover fillers,
                # then claim the next pair's attout accumulators (the ring is
                # fully drained here, so its fillers at i>=6 line up with the
                # avc evacuations exactly as in this pair) and emit its first
                # scores so its loop starts without a PE bubble.
                for e in pend:
                    e()
                for step in fill:
                    step(pool=avpool)
                nc.scalar.copy(ao[:, p, 512 * 3:], avc[3][:])
                if p < 3:
                    avc_next = [avpool.tile([P, 512], FP, tag="av",
                                            name=f"avc{p + 1}_{c}")
                                for c in range(NTG)]
                    sc0 = emit_scores(p + 1, 0)
            es.close()

            # ---- phase 3: projection tail, pipelined across both psum pools
            # with the evacuation alternating between Scalar and Vector.
            tail = [(tt, nb) for tt in range(12, NST) for nb in range(2)]
            for idx, (tt, nb) in enumerate(tail):
                proj_group(tt, nb, pool=(pspool if idx % 2 else avpool),
                           on_dve=bool(idx % 2))

    nc.compile()
    return nc


def _get_nc():
    if "nc" not in _CACHE:
        _CACHE["nc"] = _build_nc()
    return _CACHE["nc"]


def _in_maps(x, Wq, Wk, Wv, Wp):
    import ml_dtypes
    tri = np.tril(np.full((P, P), NEG, np.float32), -1)
    Wq = Wq * np.float32(QS)  # fold the score scale into q (see header)
    maps = []
    for b in range(B):
        xT = np.ascontiguousarray(x[b].T)
        for g in range(2):
            heads = range(8 * g, 8 * g + 8)
            maps.append({
                "xt": xT.astype(ml_dtypes.bfloat16),
                "wq": np.ascontiguousarray(np.concatenate([Wq[h] for h in heads], 1)).astype(ml_dtypes.bfloat16),
                "wk": np.ascontiguousarray(np.concatenate([Wk[h] for h in heads], 1)).astype(ml_dtypes.bfloat16),
                "wv": np.ascontiguousarray(np.concatenate([Wv[h] for h in heads], 1)).astype(ml_dtypes.bfloat16),
                "wpt": np.ascontiguousarray(Wp[:, 512 * g:512 * g + 512].T).astype(ml_dtypes.bfloat16),
                "tri": tri,
            })
    return maps


def kernel(x, Wq, Wk, Wv, Wp, bp):
    from concourse.bass_utils import run_bass_kernel_spmd

    x = np.asarray(x, np.float32)
    Wq = np.asarray(Wq, np.float32)
    Wk = np.asarray(Wk, np.float32)
    Wv = np.asarray(Wv, np.float32)
    Wp = np.asarray(Wp, np.float32)
    bp = np.asarray(bp, np.float32)

    nc = _get_nc()
    res = run_bass_kernel_spmd(nc, _in_maps(x, Wq, Wk, Wv, Wp), list(range(8)))
    y = np.empty((B, T, C), np.float32)
    for b in range(B):
        y[b] = res.results[2 * b]["y"] + res.results[2 * b + 1]["y"] + bp
    return y


# revision 27
# speedup vs baseline: 1.0191x; 1.0191x over previous
"""Multi-head attention (query-axis softmax variant) on 8 Trainium2 NeuronCores.

Problem: B=4, T=2048, C=1024, H=16, Dh=64.
  q/k/v = per-head projections of x; wei = (q k^T) * C**-0.5, causal-masked;
  softmax over the QUERY axis (axis=2 of (B,H,T,S)); out = attn @ v, concat
  heads, project with Wp and add bp.

Sharding: 8 cores = 4 batches x 2 head-groups (8 heads each).  Each core
computes a partial projection output for its batch; host sums the two
group partials per batch and adds the bias.

Per-core dataflow is fully "transposed" (features on partitions, tokens on
the free axis) so the query-axis softmax stats become free-axis reductions:
  xT (C, T) -> qT/kT per head-pair (128, T) -> scores W[s,t] per key-tile
  -> P = exp(W*scale) with the masked entries driven to 0 via a -1e30
  additive triangle, Z[s] = accumulated row sums from the Exp activation
  -> v' = v * (1/Z) -> attout^T[d,t] -> y = attout^T.T @ WpT.

All matmuls run in bf16.  The two heads of a pair are interleaved at chunk
granularity: score matmuls alternate PE row groups (64-row halves) and
attout matmuls alternate column groups, so each chunk's pair of matmuls
executes concurrently in the PE array and the next LDWEIGHTS pulls ahead
under the other group's matmul.  The attention loop is software-pipelined
(scores one iteration ahead, attout two behind) with the q/k projections of
the next pair, the v projection and output-projection groups interleaved as
PE filler.  Softmax stats are batched per iteration: one [P,4] accum tile,
one gpsimd add + one DVE reciprocal for both heads, v*(1/Z) on gpsimd.
"""
import numpy as np

T = 2048
C = 1024
H = 16
DH = 64
B = 4
SCALE = float(C) ** -0.5
P = 128

# Wq is pre-scaled on the host by QS = SCALE/(16*sqrt2), so the score psum
# holds x' = QS * (q.k).  exp(SCALE*q.k) is then
#   ACT path:  exp(x' * 16*sqrt2)
#   DVE path:  ((x' + 1/sqrt2)^2 + 1/2)^16    [exp16 poly, see below]
# The causal mask adds -1/sqrt2 to x' (i.e. raw score -512): ACT gives
# e^-16 ~ 1e-7, the poly hits its global minimum (1/2)^16 ~ 1.5e-5 -- both
# effectively zero, and the poly cannot overflow (a -1e30 mask would).
import math as _math
QS = SCALE / (16.0 * _math.sqrt(2.0))
ACT_SCALE = 16.0 * _math.sqrt(2.0)
NEG = -1.0 / _math.sqrt(2.0)

_CACHE = {}


def _register_exp16():
    """Register a runtime custom-DVE op computing exp-of-score with a Z
    accumulator, so the Vector engine can take half the softmax exps.

    exp(y) ~ (1 + b + b^2/2)^16 with b = y/16, via the identity
    1 + b + b^2/2 = ((b+1)/sqrt2)^2 + 1/2.  With the input pre-scaled to
    x' = y/(16*sqrt2) this is 7 ALU stages + 1 accum stage:
      t = x' + C0;  v = t^2 + C1;  out = (((v^2)^2)^2)^2
    with C0 = 1/sqrt2, C1 = 1/2.  Max relative error ~0.22% over the
    realized score range |y|<1.5, inside the bf16 output quantization."""
    import concourse.dve_ops as dve_ops
    from concourse.dve_spec import Spec, Src0, C0, C1, sq, lower
    from concourse.dve_uop import DveOpSpec
    from operator import add as _add

    for op in dve_ops.OPS:
        if op.name == "EXP16_ANT":
            return op
    body = sq(sq(sq(sq(sq(Src0 + C0) + C1))))
    spec = Spec(body=body, accum=_add)
    shas = {}
    for ver in ("v3", "v4"):
        tmp = DveOpSpec(name="EXP16_ANT", uops=lower(spec, ver=ver), rd1_en=False)
        shas[ver] = tmp.sha(ver)
    op = dve_ops.DveOp("EXP16_ANT", spec, False, shas)
    dve_ops.OPS.append(op)
    dve_ops._SUB_OPCODE_FOR_NAME[op.name] = (
        dve_ops._CUSTOM_DVE_ROW_BASE + len(dve_ops.OPS) - 1)
    dve_ops.CUSTOM_DVE_SPECS[op.name] = spec
    return op


def _build_nc():
    import concourse.bacc as bacc
    import concourse.tile as tile
    import concourse.mybir as mybir
    from contextlib import ExitStack

    FP = mybir.dt.float32
    BF = mybir.dt.bfloat16
    EXP = mybir.ActivationFunctionType.Exp
    EXP16 = _register_exp16()
    E16_S0 = 1.0 / _math.sqrt(2.0)
    E16_S1 = 0.5

    nc = bacc.Bacc("TRN2", target_bir_lowering=False, debug=False, num_devices=8)

    xT_d = nc.declare_dram_parameter("xt", [C, T], BF, isOutput=False)
    wq_d = nc.declare_dram_parameter("wq", [C, 512], BF, isOutput=False)
    wk_d = nc.declare_dram_parameter("wk", [C, 512], BF, isOutput=False)
    wv_d = nc.declare_dram_parameter("wv", [C, 512], BF, isOutput=False)
    wp_d = nc.declare_dram_parameter("wpt", [512, C], BF, isOutput=False)
    tri_d = nc.declare_dram_parameter("tri", [P, P], FP, isOutput=False)
    y_d = nc.declare_dram_parameter("y", [T, C], FP, isOutput=True)

    NCT = C // P      # 8 c-tiles
    NST = T // P      # 16 s-tiles
    NTG = T // 512    # 4 t-groups

    with tile.TileContext(nc) as tc:
        with (
            tc.tile_pool(name="perm", bufs=1) as perm,
            tc.tile_pool(name="work", bufs=4) as work,
            tc.tile_pool(name="stat", bufs=4) as stat,
            tc.tile_pool(name="statv", bufs=4) as statv,
            tc.tile_pool(name="pao", bufs=1) as pao,
            tc.tile_pool(name="ps", bufs=2, space="PSUM") as pspool,
            tc.tile_pool(name="avps", bufs=4, space="PSUM") as avpool,
        ):
            tri = perm.tile([P, P], FP, tag="tri")
            nc.sync.dma_start(tri[:], tri_d[:])
            v_sb = perm.tile([P, NST, 512], BF, tag="v")
            q_sb = perm.tile([P, 4, T], BF, tag="q")
            k_sb = perm.tile([P, 4, T], BF, tag="k")

            es = ExitStack()
            pxw = es.enter_context(tc.tile_pool(name="px", bufs=1))
            wpool = es.enter_context(tc.tile_pool(name="w", bufs=2))

            # warm tile + early activation-table load (overlaps input DMAs)
            warm = perm.tile([P, 512], BF, tag="warm")
            nc.vector.memset(warm[:], 0.0)
            wexp = perm.tile([P, 1], BF, tag="wexp")
            nc.scalar.activation(wexp[:], warm[:, 0:1], EXP, scale=SCALE)

            # ---- phase 1: qT/kT per pair (128 = [h0 d, h1 d], T), bf16 out ----
            xT = pxw.tile([P, NCT, T], BF, tag="xT")
            wv = pxw.tile([P, NCT, 512], BF, tag="wv")

            def emit_qk_steps(p):
                """Returns a list of closures; each emits one 512-col psum group."""
                wqt = wpool.tile([P, NCT, P], BF, tag="wq")
                wkt = wpool.tile([P, NCT, P], BF, tag="wk")
                nc.sync.dma_start(
                    wqt[:], wq_d.ap()[:, P * p:P * p + P].rearrange("(a c) m -> c a m", c=P))
                nc.sync.dma_start(
                    wkt[:], wk_d.ap()[:, P * p:P * p + P].rearrange("(a c) m -> c a m", c=P))
                steps = []
                for g in range(NTG):
                    for wt, dst in ((wqt, q_sb), (wkt, k_sb)):
                        def step(wt=wt, dst=dst, g=g, p=p, pool=None):
                            pl = pool if pool is not None else pspool
                            psq = pl.tile([P, 512], FP, tag="av" if pool is not None else "ps",
                                          name=f"qkps{p}_{g}_{dst is k_sb}")
                            for ct in range(NCT):
                                nc.tensor.matmul(
                                    psq[:, :512], lhsT=wt[:, ct, :],
                                    rhs=xT[:, ct, 512 * g:512 * g + 512],
                                    start=(ct == 0), stop=(ct == NCT - 1))
                            nc.scalar.copy(dst[:, p, 512 * g:512 * g + 512], psq[:, :512])
                        steps.append(step)
                return steps

            # DMA priority: pair-0 q/k weights first (the prefix needs them
            # immediately), then xT by t-group with the v weights mid-stream.
            steps0 = emit_qk_steps(0)
            for g in range(NTG):
                # per (t-group, c-half) so the accumulation matmuls can pace
                # with the DMA arrivals at half-group granularity
                for ch in range(2):
                    nc.sync.dma_start(
                        xT[:, 4 * ch:4 * ch + 4, 512 * g:512 * g + 512],
                        xT_d.ap()[512 * ch:512 * ch + 512,
                                  512 * g:512 * g + 512].rearrange(
                            "(a c) t -> c a t", c=P))
                if g == 1:
                    nc.sync.dma_start(wv[:], wv_d.ap().rearrange("(a c) m -> c a m", c=P))

            # Warm up the PE's HAM clock gate while the input DMAs land (kept
            # short: these sit ahead of the real work in the in-order queue).
            for wi in range(8):
                wps = pspool.tile([P, 1024], FP, tag="ps", name=f"warm{wi}")
                for _ in range(2):
                    nc.tensor.matmul(wps[:, :512], lhsT=warm[:, :P], rhs=warm[:],
                                     start=True, stop=True)

            def v_step(st, pool=None):
                pl = pool if pool is not None else pspool
                ps = pl.tile([P, 512] if pool is not None else [P, 1024], FP,
                             tag="av" if pool is not None else "ps", name=f"vps{st}")
                for ct in range(NCT):
                    nc.tensor.matmul(
                        ps[:, :512],
                        lhsT=xT[:, ct, P * st:P * st + P],
                        rhs=wv[:, ct, :],
                        start=(ct == 0), stop=(ct == NCT - 1))
                nc.scalar.copy(v_sb[:, st, :], ps[:, :512])

            # ---- serial prefix: pair-0 q/k projections.  scores(0,0) is
            # emitted after the first four groups (its block A only needs
            # q groups 0-1 and k group 0) so the exps start ~9us earlier;
            # the remaining q groups come before k groups 2-3 (block B of
            # scores 0-1 needs q, while k tiles 2-3 are only read at i>=8).
            # ---- phase 2: attention ----
            ao = pao.tile([P, 4, T], BF, tag="ao")

            late_sps = {}

            def emit_scores(p, i):
                """PE: both heads' score matmuls, chunk-interleaved so the two
                row-group halves run concurrently; DVE: diag masks.

                For i>=12 the block is <=512 cols, so iterations (12,13) and
                (14,15) share one [P,1024] tile per head at column offsets
                0/512: scores(i+1) then never waits on exp(i) (different psum
                banks of the already-granted slot).  For i>=8 the h1 chunk is
                emitted first: its slot is freed by the DVE exp16, which
                finishes earlier than the ACT exp that frees h0's slot."""
                t0 = P * i
                blocks = [(t0, 1024), (1024, 2048)] if i < 8 else [(t0, 2048)]
                prows = [work.tile([P, T], BF, tag="prow", bufs=8,
                                   name=f"prow{p}_{i}_{h}") for h in range(2)]
                zps = stat.tile([P, 4], FP, tag="zp", bufs=8, name=f"zp{p}_{i}")
                hls = (0, 1)
                tiles = []
                for bi, (lo, hi) in enumerate(blocks):
                    base = [pspool.tile([P, 1024], FP, tag="ps",
                                        name=f"sps{p}_{i}_{bi}_{h}") for h in range(2)]
                    off = 0
                    for clo in range(lo, hi, 512):
                        chi = min(clo + 512, hi)
                        for hl in hls:
                            hb = 64 * hl
                            nc.tensor.matmul(
                                base[hl][:, off + clo - lo:off + chi - lo],
                                lhsT=k_sb[hb:hb + 64, p, t0:t0 + P],
                                rhs=q_sb[hb:hb + 64, p, clo:chi],
                                start=True, stop=True)
                        if clo == t0:
                            for hl in hls:
                                nc.vector.tensor_add(
                                    base[hl][:, off:off + P],
                                    base[hl][:, off:off + P], tri[:])
                    tiles.append((base, off, bi, lo, hi))
                return dict(i=i, t0=t0, nb=len(blocks), prows=prows, zps=zps, tiles=tiles)

            def emit_exps(sc):
                # Engine split chosen so the two psum ring slots (slot A = h0,
                # slot B = h1) free at ~the same time: each slot gets one ACT
                # and one DVE block when there are two blocks, and the two
                # heads go to different engines when there is one.
                for (base, off, bi, lo, hi) in sc["tiles"]:
                    for hl in range(2):
                        on_dve = (hl ^ bi) if sc["nb"] == 2 else hl
                        zcol = sc["zps"][:, 2 * bi + hl:2 * bi + hl + 1]
                        if on_dve:
                            nc.vector._custom_dve(
                                EXP16, out=sc["prows"][hl][:, lo:hi],
                                in0=base[hl][:, off:off + hi - lo],
                                s0=E16_S0, s1=E16_S1, imm2=0.0, accum_out=zcol)
                        else:
                            nc.scalar.activation(
                                sc["prows"][hl][:, lo:hi],
                                base[hl][:, off:off + hi - lo],
                                EXP, scale=ACT_SCALE, accum_out=zcol)

            def emit_stats(p, sc):
                i = sc["i"]
                zps = sc["zps"]
                if sc["nb"] == 2:
                    zz = stat.tile([P, 2], FP, tag="z", name=f"z{p}_{i}")
                    nc.gpsimd.tensor_add(zz[:], zps[:, 0:2], zps[:, 2:4])
                    zzap = zz[:]
                else:
                    zzap = zps[:, 0:2]
                rz = stat.tile([P, 2], FP, tag="rz", name=f"rz{p}_{i}")
                nc.vector.reciprocal(rz[:], zzap)
                vps = []
                for hl in range(2):
                    vp = statv.tile([P, 64], BF, tag="vp", bufs=6, name=f"vp{p}_{i}_{hl}")
                    hh = 64 * (2 * p + hl)
                    nc.vector.tensor_scalar_mul(
                        vp[:], v_sb[:, i, hh:hh + 64], rz[:, hl:hl + 1])
                    vps.append(vp)
                return vps

            wpt = pao.tile([P, 4, C], BF, tag="wpt")
            nc.sync.dma_start(wpt[:], wp_d.ap().rearrange("(a c) m -> c a m", c=P))

            def proj_group(tt, nb, pool=None, on_dve=False):
                pl = pool if pool is not None else avpool
                ps = pl.tile([P, 512], FP, tag="av" if pl is avpool else "ps",
                             name=f"pps{tt}_{nb}")
                for pp in range(4):
                    nc.tensor.matmul(
                        ps[:, :512], lhsT=ao[:, pp, P * tt:P * tt + P],
                        rhs=wpt[:, pp, 512 * nb:512 * nb + 512],
                        start=(pp == 0), stop=(pp == 3))
                yt = work.tile([P, 512], FP, tag="yt", bufs=4, name=f"yt{tt}_{nb}")
                if on_dve:
                    nc.vector.tensor_copy(yt[:], ps[:, :512])
                else:
                    nc.scalar.copy(yt[:], ps[:, :512])
                nc.sync.dma_start(y_d.ap()[P * tt:P * tt + P, 512 * nb:512 * nb + 512], yt[:])

            for si in (0, 1, 2, 3, 4, 6):
                steps0[si]()
            # all q groups are now emitted, so scores(0,0) [needs q 0-3 and
            # k group 0] can be emitted; k groups 2-3 (read at i>=8) follow.
            sc00 = emit_scores(0, 0)
            for si in (5, 7):
                steps0[si]()

            sc0 = sc00
            avc_next = None
            for p in range(4):
                if p == 0:
                    fill = [(lambda st=st: (lambda pool=None: v_step(st, pool=pool)))()
                            for st in range(8, NST)] + emit_qk_steps(1)
                elif p < 3:
                    fill = emit_qk_steps(p + 1)
                else:
                    fill = [(lambda tt=tt, nb=nb: (lambda pool=None: proj_group(tt, nb)))()
                            for tt in range(12) for nb in range(2)]
                if avc_next is None:
                    avc = [avpool.tile([P, 512], FP, tag="av", name=f"avc{p}_{c}")
                           for c in range(NTG)]
                else:
                    avc = avc_next
                avc_next = None
                pend = []
                done_av = -1
                evacd = 0
                sc = sc0 if sc0 is not None else emit_scores(p, 0)
                for i in range(NST):
                    sc_next = emit_scores(p, i + 1) if i < NST - 1 else None
                    emit_exps(sc)
                    if len(pend) >= 2 or (i == NST - 1 and pend):
                        pend.pop(0)()
                        done_av += 1
                    # evacuate finished attout chunks (frees their psum bank
                    # for the qk / projection filler steps); alternate the
                    # copy engine to balance Scalar vs Vector load
                    if evacd < 3 and done_av == 4 * evacd + 3:
                        if evacd % 2:
                            nc.scalar.copy(
                                ao[:, p, 512 * evacd:512 * evacd + 512], avc[evacd][:])
                        else:
                            nc.vector.tensor_copy(
                                ao[:, p, 512 * evacd:512 * evacd + 512], avc[evacd][:])
                        evacd += 1
                    if p == 0 and i < 4:
                        v_step(2 * i)
                        v_step(2 * i + 1)
                    if fill and i >= 6:
                        n = 2 if len(fill) > (NST - 1 - i) else 1
                        for _ in range(min(n, len(fill))):
                            fill.pop(0)(pool=avpool)
                    vps = emit_stats(p, sc)

                    def make_av(i=i, t0=P * i, vps=vps, prows=sc["prows"]):
                        def emit():
                            for c in range(NTG):
                                clo, chi = 512 * c, 512 * c + 512
                                lo2 = max(clo, t0)
                                if lo2 >= chi:
                                    continue
                                for hl in range(2):
                                    hb = 64 * hl
                                    nc.tensor.matmul(
                                        avc[c][hb:hb + 64, lo2 - clo:512],
                                        lhsT=vps[hl][:], rhs=prows[hl][:, lo2:chi],
                                        start=(i == 0), stop=(i == 4 * c + 3))
                        return emit

                    pend.append(make_av())
                    sc = sc_next
                # pair tail: drain attout (ready soonest) and leftover fillers,
                # then claim the next pair's attout accumulators (the ring is
                # fully drained here, so its fillers at i>=6 line up with the
                # avc evacuations exactly as in this pair) and emit its first
                # scores so its loop starts without a PE bubble.
                for e in pend:
                    e()
                for step in fill:
                    step(pool=avpool)
                nc.scalar.copy(ao[:, p, 512 * 3:], avc[3][:])
                if p < 3:
                    avc_next = [avpool.tile([P, 512], FP, tag="av",
                                            name=f"avc{p + 1}_{c}")
                                for c in range(NTG)]
                    sc0 = emit_scores(p + 1, 0)
            es.close()

            # ---- phase 3: projection tail, pipelined across both psum pools
            # with the evacuation alternating between Scalar and Vector.
            tail = [(tt, nb) for tt in range(12, NST) for nb in range(2)]
            for idx, (tt, nb) in enumerate(tail):
                proj_group(tt, nb, pool=(pspool if idx % 2 else avpool),
                           on_dve=bool(idx % 2))

    nc.compile()
    return nc


def _get_nc():
    if "nc" not in _CACHE:
        _CACHE["nc"] = _build_nc()
    return _CACHE["nc"]


def _in_maps(x, Wq, Wk, Wv, Wp):
    import ml_dtypes
    tri = np.tril(np.full((P, P), NEG, np.float32), -1)
    Wq = Wq * np.float32(QS)  # fold the score scale into q (see header)
    maps = []
    for b in range(B):
        xT = np.ascontiguousarray(x[b].T)
        for g in range(2):
            heads = range(8 * g, 8 * g + 8)
            maps.append({
                "xt": xT.astype(ml_dtypes.bfloat16),
                "wq": np.ascontiguousarray(np.concatenate([Wq[h] for h in heads], 1)).astype(ml_dtypes.bfloat16),
                "wk": np.ascontiguousarray(np.concatenate([Wk[h] for h in heads], 1)).astype(ml_dtypes.bfloat16),
                "wv": np.ascontiguousarray(np.concatenate([Wv[h] for h in heads], 1)).astype(ml_dtypes.bfloat16),
                "wpt": np.ascontiguousarray(Wp[:, 512 * g:512 * g + 512].T).astype(ml_dtypes.bfloat16),
                "tri": tri,
            })
    return maps


def kernel(x, Wq, Wk, Wv, Wp, bp):
    from concourse.bass_utils import run_bass_kernel_spmd

    x = np.asarray(x, np.float32)
    Wq = np.asarray(Wq, np.float32)
    Wk = np.asarray(Wk, np.float32)
    Wv = np.asarray(Wv, np.float32)
    Wp = np.asarray(Wp, np.float32)
    bp = np.asarray(bp, np.float32)

    nc = _get_nc()
    res = run_bass_kernel_spmd(nc, _in_maps(x, Wq, Wk, Wv, Wp), list(range(8)))
    y = np.empty((B, T, C), np.float32)
    for b in range(B):
        y[b] = res.results[2 * b]["y"] + res.results[2 * b + 1]["y"] + bp
    return y


# revision 31
# speedup vs baseline: 1.0598x; 1.0399x over previous
"""Multi-head attention (query-axis softmax variant) on 8 Trainium2 NeuronCores.

Problem: B=4, T=2048, C=1024, H=16, Dh=64.
  q/k/v = per-head projections of x; wei = (q k^T) * C**-0.5, causal-masked;
  softmax over the QUERY axis (axis=2 of (B,H,T,S)); out = attn @ v, concat
  heads, project with Wp and add bp.

Sharding: 8 cores = 4 batches x 2 head-groups (8 heads each).  Each core
computes a partial projection output for its batch; host sums the two
group partials per batch and adds the bias.

Per-core dataflow is fully "transposed" (features on partitions, tokens on
the free axis) so the query-axis softmax stats become free-axis reductions:
  xT (C, T) -> qT/kT per head-pair (128, T) -> scores W[s,t] per key-tile
  -> P = exp(W*scale) with the masked entries driven to 0 via a -1e30
  additive triangle, Z[s] = accumulated row sums from the Exp activation
  -> v' = v * (1/Z) -> attout^T[d,t] -> y = attout^T.T @ WpT.

All matmuls run in bf16.  The two heads of a pair are interleaved at chunk
granularity: score matmuls alternate PE row groups (64-row halves) and
attout matmuls alternate column groups, so each chunk's pair of matmuls
executes concurrently in the PE array and the next LDWEIGHTS pulls ahead
under the other group's matmul.  The attention loop is software-pipelined
(scores one iteration ahead, attout two behind) with the q/k projections of
the next pair, the v projection and output-projection groups interleaved as
PE filler.  Softmax stats are batched per iteration: one [P,4] accum tile,
one gpsimd add + one DVE reciprocal for both heads, v*(1/Z) on gpsimd.
"""
import numpy as np

T = 2048
C = 1024
H = 16
DH = 64
B = 4
SCALE = float(C) ** -0.5
P = 128

# Wq is pre-scaled on the host by QS = SCALE/(16*sqrt2), so the score psum
# holds x' = QS * (q.k).  exp(SCALE*q.k) is then
#   ACT path:  exp(x' * 16*sqrt2)
#   DVE path:  ((x' + 1/sqrt2)^2 + 1/2)^16    [exp16 poly, see below]
# The causal mask adds -1/sqrt2 to x' (i.e. raw score -512): ACT gives
# e^-16 ~ 1e-7, the poly hits its global minimum (1/2)^16 ~ 1.5e-5 -- both
# effectively zero, and the poly cannot overflow (a -1e30 mask would).
import math as _math
QS = SCALE / (16.0 * _math.sqrt(2.0))
ACT_SCALE = 16.0 * _math.sqrt(2.0)
NEG = -1.0 / _math.sqrt(2.0)

_CACHE = {}


def _register_exp16():
    """Register a runtime custom-DVE op computing exp-of-score with a Z
    accumulator, so the Vector engine can take half the softmax exps.

    exp(y) ~ (1 + b + b^2/2)^16 with b = y/16, via the identity
    1 + b + b^2/2 = ((b+1)/sqrt2)^2 + 1/2.  With the input pre-scaled to
    x' = y/(16*sqrt2) this is 7 ALU stages + 1 accum stage:
      t = x' + C0;  v = t^2 + C1;  out = (((v^2)^2)^2)^2
    with C0 = 1/sqrt2, C1 = 1/2.  Max relative error ~0.22% over the
    realized score range |y|<1.5, inside the bf16 output quantization."""
    import concourse.dve_ops as dve_ops
    from concourse.dve_spec import Spec, Src0, C0, C1, sq, lower
    from concourse.dve_uop import DveOpSpec
    from operator import add as _add

    for op in dve_ops.OPS:
        if op.name == "EXP16_ANT":
            return op
    body = sq(sq(sq(sq(sq(Src0 + C0) + C1))))
    spec = Spec(body=body, accum=_add)
    shas = {}
    for ver in ("v3", "v4"):
        tmp = DveOpSpec(name="EXP16_ANT", uops=lower(spec, ver=ver), rd1_en=False)
        shas[ver] = tmp.sha(ver)
    op = dve_ops.DveOp("EXP16_ANT", spec, False, shas)
    dve_ops.OPS.append(op)
    dve_ops._SUB_OPCODE_FOR_NAME[op.name] = (
        dve_ops._CUSTOM_DVE_ROW_BASE + len(dve_ops.OPS) - 1)
    dve_ops.CUSTOM_DVE_SPECS[op.name] = spec
    return op


def _build_nc():
    import concourse.bacc as bacc
    import concourse.tile as tile
    import concourse.mybir as mybir
    from contextlib import ExitStack

    FP = mybir.dt.float32
    BF = mybir.dt.bfloat16
    EXP = mybir.ActivationFunctionType.Exp
    EXP16 = _register_exp16()
    E16_S0 = 1.0 / _math.sqrt(2.0)
    E16_S1 = 0.5

    nc = bacc.Bacc("TRN2", target_bir_lowering=False, debug=False, num_devices=8)

    xT_d = nc.declare_dram_parameter("xt", [C, T], BF, isOutput=False)
    wq_d = nc.declare_dram_parameter("wq", [C, 512], BF, isOutput=False)
    wk_d = nc.declare_dram_parameter("wk", [C, 512], BF, isOutput=False)
    wv_d = nc.declare_dram_parameter("wv", [C, 512], BF, isOutput=False)
    wp_d = nc.declare_dram_parameter("wpt", [512, C], BF, isOutput=False)
    tri_d = nc.declare_dram_parameter("tri", [P, P], FP, isOutput=False)
    y_d = nc.declare_dram_parameter("y", [T, C], FP, isOutput=True)

    NCT = C // P      # 8 c-tiles
    NST = T // P      # 16 s-tiles
    NTG = T // 512    # 4 t-groups

    with tile.TileContext(nc) as tc:
        with (
            tc.tile_pool(name="perm", bufs=1) as perm,
            tc.tile_pool(name="work", bufs=4) as work,
            tc.tile_pool(name="stat", bufs=4) as stat,
            tc.tile_pool(name="statv", bufs=4) as statv,
            tc.tile_pool(name="pao", bufs=1) as pao,
            tc.tile_pool(name="ps", bufs=2, space="PSUM") as pspool,
            tc.tile_pool(name="avps", bufs=4, space="PSUM") as avpool,
        ):
            tri = perm.tile([P, P], FP, tag="tri")
            nc.sync.dma_start(tri[:], tri_d[:])
            v_sb = perm.tile([P, NST, 512], BF, tag="v")
            q_sb = perm.tile([P, 4, T], BF, tag="q")
            k_sb = perm.tile([P, 4, T], BF, tag="k")

            es = ExitStack()
            pxw = es.enter_context(tc.tile_pool(name="px", bufs=1))
            wpool = es.enter_context(tc.tile_pool(name="w", bufs=2))

            # warm tile + early activation-table load (overlaps input DMAs)
            warm = perm.tile([P, 512], BF, tag="warm")
            nc.vector.memset(warm[:], 0.0)
            wexp = perm.tile([P, 1], BF, tag="wexp")
            nc.scalar.activation(wexp[:], warm[:, 0:1], EXP, scale=SCALE)

            # ---- phase 1: qT/kT per pair (128 = [h0 d, h1 d], T), bf16 out ----
            xT = pxw.tile([P, NCT, T], BF, tag="xT")
            wv = pxw.tile([P, NCT, 512], BF, tag="wv")

            def emit_qk_steps(p):
                """Returns a list of closures; each emits one 512-col psum group."""
                wqt = wpool.tile([P, NCT, P], BF, tag="wq")
                wkt = wpool.tile([P, NCT, P], BF, tag="wk")
                nc.sync.dma_start(
                    wqt[:], wq_d.ap()[:, P * p:P * p + P].rearrange("(a c) m -> c a m", c=P))
                nc.sync.dma_start(
                    wkt[:], wk_d.ap()[:, P * p:P * p + P].rearrange("(a c) m -> c a m", c=P))
                steps = []
                for g in range(NTG):
                    for si, (wt, dst) in enumerate(((wqt, q_sb), (wkt, k_sb))):
                        def step(wt=wt, dst=dst, g=g, p=p, si=si, pool=None):
                            pl = pool if pool is not None else pspool
                            psq = pl.tile([P, 512], FP, tag="av" if pool is not None else "ps",
                                          name=f"qkps{p}_{g}_{dst is k_sb}")
                            for ct in range(NCT):
                                nc.tensor.matmul(
                                    psq[:, :512], lhsT=wt[:, ct, :],
                                    rhs=xT[:, ct, 512 * g:512 * g + 512],
                                    start=(ct == 0), stop=(ct == NCT - 1))
                            # alternate the evacuation engine so a busy queue
                            # on one engine doesn't delay every slot turnover
                            if (g + si) % 2:
                                nc.vector.tensor_copy(
                                    dst[:, p, 512 * g:512 * g + 512], psq[:, :512])
                            else:
                                nc.scalar.copy(
                                    dst[:, p, 512 * g:512 * g + 512], psq[:, :512])
                        steps.append(step)
                return steps

            # DMA priority: pair-0 q/k weights first (the prefix needs them
            # immediately), then xT by t-group with the v weights mid-stream.
            steps0 = emit_qk_steps(0)
            for g in range(NTG):
                # per (t-group, c-quarter) so the accumulation matmuls can
                # pace with the DMA arrivals at fine granularity
                for ch in range(4):
                    nc.sync.dma_start(
                        xT[:, 2 * ch:2 * ch + 2, 512 * g:512 * g + 512],
                        xT_d.ap()[256 * ch:256 * ch + 256,
                                  512 * g:512 * g + 512].rearrange(
                            "(a c) t -> c a t", c=P))
                if g == 1:
                    nc.sync.dma_start(wv[:], wv_d.ap().rearrange("(a c) m -> c a m", c=P))

            # Warm up the PE's HAM clock gate while the input DMAs land (kept
            # short: these sit ahead of the real work in the in-order queue).
            for wi in range(8):
                wps = pspool.tile([P, 1024], FP, tag="ps", name=f"warm{wi}")
                for _ in range(2):
                    nc.tensor.matmul(wps[:, :512], lhsT=warm[:, :P], rhs=warm[:],
                                     start=True, stop=True)

            def v_step(st, pool=None):
                pl = pool if pool is not None else pspool
                ps = pl.tile([P, 512] if pool is not None else [P, 1024], FP,
                             tag="av" if pool is not None else "ps", name=f"vps{st}")
                for ct in range(NCT):
                    nc.tensor.matmul(
                        ps[:, :512],
                        lhsT=xT[:, ct, P * st:P * st + P],
                        rhs=wv[:, ct, :],
                        start=(ct == 0), stop=(ct == NCT - 1))
                if st % 2:
                    nc.vector.tensor_copy(v_sb[:, st, :], ps[:, :512])
                else:
                    nc.scalar.copy(v_sb[:, st, :], ps[:, :512])

            # ---- serial prefix: pair-0 q/k projections.  scores(0,0) is
            # emitted after the first four groups (its block A only needs
            # q groups 0-1 and k group 0) so the exps start ~9us earlier;
            # the remaining q groups come before k groups 2-3 (block B of
            # scores 0-1 needs q, while k tiles 2-3 are only read at i>=8).
            # ---- phase 2: attention ----
            ao = pao.tile([P, 4, T], BF, tag="ao")

            def emit_scores(p, i):
                """PE: both heads' score matmuls, chunk-interleaved so the two
                row-group halves run concurrently; DVE: diag masks."""
                t0 = P * i
                blocks = [(t0, 1024), (1024, 2048)] if i < 8 else [(t0, 2048)]
                prows = [work.tile([P, T], BF, tag="prow", bufs=8,
                                   name=f"prow{p}_{i}_{h}") for h in range(2)]
                zps = stat.tile([P, 4], FP, tag="zp", bufs=8, name=f"zp{p}_{i}")
                hls = (0, 1)
                tiles = []
                for bi, (lo, hi) in enumerate(blocks):
                    base = [pspool.tile([P, 1024], FP, tag="ps",
                                        name=f"sps{p}_{i}_{bi}_{h}") for h in range(2)]
                    off = 0
                    for clo in range(lo, hi, 512):
                        chi = min(clo + 512, hi)
                        for hl in hls:
                            hb = 64 * hl
                            nc.tensor.matmul(
                                base[hl][:, off + clo - lo:off + chi - lo],
                                lhsT=k_sb[hb:hb + 64, p, t0:t0 + P],
                                rhs=q_sb[hb:hb + 64, p, clo:chi],
                                start=True, stop=True)
                        if clo == t0:
                            for hl in hls:
                                nc.vector.tensor_add(
                                    base[hl][:, off:off + P],
                                    base[hl][:, off:off + P], tri[:])
                    tiles.append((base, off, bi, lo, hi))
                return dict(i=i, t0=t0, nb=len(blocks), prows=prows, zps=zps, tiles=tiles)

            def emit_exps(sc):
                # Engine split chosen so the two psum ring slots (slot A = h0,
                # slot B = h1) free at ~the same time: each slot gets one ACT
                # and one DVE block when there are two blocks, and the two
                # heads go to different engines when there is one.
                for (base, off, bi, lo, hi) in sc["tiles"]:
                    for hl in range(2):
                        on_dve = (hl ^ bi) if sc["nb"] == 2 else hl
                        zcol = sc["zps"][:, 2 * bi + hl:2 * bi + hl + 1]
                        if on_dve:
                            nc.vector._custom_dve(
                                EXP16, out=sc["prows"][hl][:, lo:hi],
                                in0=base[hl][:, off:off + hi - lo],
                                s0=E16_S0, s1=E16_S1, imm2=0.0, accum_out=zcol)
                        else:
                            nc.scalar.activation(
                                sc["prows"][hl][:, lo:hi],
                                base[hl][:, off:off + hi - lo],
                                EXP, scale=ACT_SCALE, accum_out=zcol)

            def emit_stats(p, sc):
                i = sc["i"]
                zps = sc["zps"]
                if sc["nb"] == 2:
                    zz = stat.tile([P, 2], FP, tag="z", name=f"z{p}_{i}")
                    nc.gpsimd.tensor_add(zz[:], zps[:, 0:2], zps[:, 2:4])
                    zzap = zz[:]
                else:
                    zzap = zps[:, 0:2]
                rz = stat.tile([P, 2], FP, tag="rz", name=f"rz{p}_{i}")
                nc.vector.reciprocal(rz[:], zzap)
                vps = []
                for hl in range(2):
                    vp = statv.tile([P, 64], BF, tag="vp", bufs=6, name=f"vp{p}_{i}_{hl}")
                    hh = 64 * (2 * p + hl)
                    nc.vector.tensor_scalar_mul(
                        vp[:], v_sb[:, i, hh:hh + 64], rz[:, hl:hl + 1])
                    vps.append(vp)
                return vps

            wpt = pao.tile([P, 4, C], BF, tag="wpt")
            nc.sync.dma_start(wpt[:], wp_d.ap().rearrange("(a c) m -> c a m", c=P))

            def proj_group(tt, nb, pool=None, on_dve=False):
                pl = pool if pool is not None else avpool
                ps = pl.tile([P, 512], FP, tag="av" if pl is avpool else "ps",
                             name=f"pps{tt}_{nb}")
                for pp in range(4):
                    nc.tensor.matmul(
                        ps[:, :512], lhsT=ao[:, pp, P * tt:P * tt + P],
                        rhs=wpt[:, pp, 512 * nb:512 * nb + 512],
                        start=(pp == 0), stop=(pp == 3))
                yt = work.tile([P, 512], FP, tag="yt", bufs=4, name=f"yt{tt}_{nb}")
                if on_dve:
                    nc.vector.tensor_copy(yt[:], ps[:, :512])
                else:
                    nc.scalar.copy(yt[:], ps[:, :512])
                nc.sync.dma_start(y_d.ap()[P * tt:P * tt + P, 512 * nb:512 * nb + 512], yt[:])

            for si in (0, 1, 2, 3, 4, 6):
                steps0[si]()
            # all q groups are now emitted, so scores(0,0) [needs q 0-3 and
            # k group 0] can be emitted; k groups 2-3 (read at i>=8) follow.
            sc00 = emit_scores(0, 0)
            for si in (5, 7):
                steps0[si]()

            sc0 = sc00
            avc_next = None
            for p in range(4):
                if p == 0:
                    fill = [(lambda st=st: (lambda pool=None: v_step(st, pool=pool)))()
                            for st in range(8, NST)] + emit_qk_steps(1)
                elif p < 3:
                    fill = emit_qk_steps(p + 1)
                else:
                    fill = [(lambda tt=tt, nb=nb: (lambda pool=None: proj_group(tt, nb)))()
                            for tt in range(12) for nb in range(2)]
                if avc_next is None:
                    avc = [avpool.tile([P, 512], FP, tag="av", name=f"avc{p}_{c}")
                           for c in range(NTG)]
                else:
                    avc = avc_next
                avc_next = None
                pend = []
                done_av = -1
                evacd = 0
                sc = sc0 if sc0 is not None else emit_scores(p, 0)
                for i in range(NST):
                    sc_next = emit_scores(p, i + 1) if i < NST - 1 else None
                    emit_exps(sc)
                    if len(pend) >= 2 or (i == NST - 1 and pend):
                        pend.pop(0)()
                        done_av += 1
                    # evacuate finished attout chunks (frees their psum bank
                    # for the qk / projection filler steps); alternate the
                    # copy engine to balance Scalar vs Vector load
                    if evacd < 3 and done_av == 4 * evacd + 3:
                        if evacd % 2:
                            nc.scalar.copy(
                                ao[:, p, 512 * evacd:512 * evacd + 512], avc[evacd][:])
                        else:
                            nc.vector.tensor_copy(
                                ao[:, p, 512 * evacd:512 * evacd + 512], avc[evacd][:])
                        evacd += 1
                    if p == 0 and i < 4:
                        v_step(2 * i)
                        v_step(2 * i + 1)
                    if fill and i >= 6:
                        n = 2 if len(fill) > (NST - 1 - i) else 1
                        for _ in range(min(n, len(fill))):
                            fill.pop(0)(pool=avpool)
                    vps = emit_stats(p, sc)

                    def make_av(i=i, t0=P * i, vps=vps, prows=sc["prows"]):
                        def emit():
                            for c in range(NTG):
                                clo, chi = 512 * c, 512 * c + 512
                                lo2 = max(clo, t0)
                                if lo2 >= chi:
                                    continue
                                for hl in range(2):
                                    hb = 64 * hl
                                    nc.tensor.matmul(
                                        avc[c][hb:hb + 64, lo2 - clo:512],
                                        lhsT=vps[hl][:], rhs=prows[hl][:, lo2:chi],
                                        start=(i == 0), stop=(i == 4 * c + 3))
                        return emit

                    pend.append(make_av())
                    sc = sc_next
                # pair tail: drain attout (ready soonest) and leftover fillers,
                # then claim the next pair's attout accumulators (the ring is
                # fully drained here, so its fillers at i>=6 line up with the
                # avc evacuations exactly as in this pair) and emit its first
                # scores so its loop starts without a PE bubble.
                for e in pend:
                    e()
                for step in fill:
                    step(pool=avpool)
                nc.scalar.copy(ao[:, p, 512 * 3:], avc[3][:])
                if p < 3:
                    avc_next = [avpool.tile([P, 512], FP, tag="av",
                                            name=f"avc{p + 1}_{c}")
                                for c in range(NTG)]
                    sc0 = emit_scores(p + 1, 0)
            es.close()

            # ---- phase 3: projection tail, pipelined across both psum pools
            # with the evacuation alternating between Scalar and Vector.
            tail = [(tt, nb) for tt in range(12, NST) for nb in range(2)]
            for idx, (tt, nb) in enumerate(tail):
                proj_group(tt, nb, pool=(pspool if idx % 2 else avpool),
                           on_dve=bool(idx % 2))

    nc.compile()
    return nc


def _get_nc():
    if "nc" not in _CACHE:
        _CACHE["nc"] = _build_nc()
    return _CACHE["nc"]


def _in_maps(x, Wq, Wk, Wv, Wp):
    import ml_dtypes
    tri = np.tril(np.full((P, P), NEG, np.float32), -1)
    Wq = Wq * np.float32(QS)  # fold the score scale into q (see header)
    maps = []
    for b in range(B):
        xT = np.ascontiguousarray(x[b].T)
        for g in range(2):
            heads = range(8 * g, 8 * g + 8)
            maps.append({
                "xt": xT.astype(ml_dtypes.bfloat16),
                "wq": np.ascontiguousarray(np.concatenate([Wq[h] for h in heads], 1)).astype(ml_dtypes.bfloat16),
                "wk": np.ascontiguousarray(np.concatenate([Wk[h] for h in heads], 1)).astype(ml_dtypes.bfloat16),
                "wv": np.ascontiguousarray(np.concatenate([Wv[h] for h in heads], 1)).astype(ml_dtypes.bfloat16),
                "wpt": np.ascontiguousarray(Wp[:, 512 * g:512 * g + 512].T).astype(ml_dtypes.bfloat16),
                "tri": tri,
            })
    return maps


def kernel(x, Wq, Wk, Wv, Wp, bp):
    from concourse.bass_utils import run_bass_kernel_spmd

    x = np.asarray(x, np.float32)
    Wq = np.asarray(Wq, np.float32)
    Wk = np.asarray(Wk, np.float32)
    Wv = np.asarray(Wv, np.float32)
    Wp = np.asarray(Wp, np.float32)
    bp = np.asarray(bp, np.float32)

    nc = _get_nc()
    res = run_bass_kernel_spmd(nc, _in_maps(x, Wq, Wk, Wv, Wp), list(range(8)))
    y = np.empty((B, T, C), np.float32)
    for b in range(B):
        y[b] = res.results[2 * b]["y"] + res.results[2 * b + 1]["y"] + bp
    return y
